# revision 14
# baseline (speedup 1.0000x reference)
"""Trainium2 Bass kernel for nn_ContrastiveLossOriginal (SimCLR-style NT-Xent loss).

reference:
    z_i = l2norm(proj_1); z_j = l2norm(proj_2); reps = concat([z_i, z_j])  # [2B, D]
    sim = reps @ reps.T / temp
    pos = rowsum(z_i * z_j)
    lse = logsumexp(sim, axis=1)           (full row, diag included)
    loss = mean(-pos/temp + lse);  also returns sum(pos)

Sharding: data-parallel over the 2B=8192 rows; each of the 8 cores owns 1024
rows, computes its [1024, 8192] slice of sim via matmul against the full
normalized rep set (built redundantly per-core from the full inputs), does the
per-row exp-sum locally, and returns per-row terms.  Host sums the scalars.

Key numerics: rows are unit vectors so row-max(sim) == diag == 1.0 (Cauchy-
Schwarz).  logsumexp therefore uses a fixed shift: lse = 1/t + ln(sum exp(
sim/t - 1/t)), which the ACT engine computes fused (scale/bias + accum_out).
Matmul operands are bf16 (error ~2e-4 per diag entry -> ~3e-6 on the mean
loss); positives are computed in fp32.
"""

import math
import os

import numpy as np

import concourse.bass as bass
import concourse.bacc as bacc
import concourse.tile as tile
from concourse import mybir
from concourse.bass_utils import run_bass_kernel_spmd

F32 = mybir.dt.float32
BF16 = mybir.dt.bfloat16
AF = mybir.ActivationFunctionType
ALU = mybir.AluOpType

B = 4096           # batch per proj tensor
D = 256            # feature dim
NROWS = 2 * B      # 8192 rows of reps
NCORES = 8
LROWS = NROWS // NCORES   # 1024 local rows per core
P = 128
KH = D // P        # 2 contraction halves
MCH = LROWS // P   # 8 local M chunks of 128 rows
PSUM_W = 2048      # psum tile width (4 banks)
NBLK = NROWS // PSUM_W    # 4 psum tiles per M chunk
INV_T = 1000.0     # 1 / temperature

# groups of 128 rows
NG_FULL_HALF = B // P      # 32 row-groups per proj tensor
NG_LOC = LROWS // P        # 8 row-groups in the local slice


def _normalize_chunk(nc, pools, x, ng, lnb, scale_eng):
    """Given row-major fp32 x [128, ng, 256], return (y1 [128, ng] fp32 inverse
    norms, z [128, KH, ng, 128] bf16 normalized rows).

    Norms via bn_stats (sum(x^2) = D*(var + mean^2)); inverse sqrt via
    exp(-0.5*ln(.)) on ACT (same table set as the main-loop Exp/Ln, so no
    table switches) + one Newton step on DVE for fp32-level accuracy."""
    xin, sqp, zbf, stat = pools
    stats = sqp.tile([P, ng, 6], F32, tag="bnstats")
    for g in range(ng):
        nc.vector.bn_stats(stats[:, g, :], x[:, g, :])
    mv = stat.tile([P, ng, 2], F32, tag="mv")
    for g in range(ng):
        nc.vector.bn_aggr(mv[:, g, :], stats[:, g, :])
    m2 = stat.tile([P, ng], F32, tag="m2")
    nc.vector.tensor_mul(m2[:], mv[:, :, 0], mv[:, :, 0])
    n2m = stat.tile([P, ng], F32, tag="n2m")
    nc.vector.tensor_add(n2m[:], m2[:], mv[:, :, 1])  # E[x^2] = sum(x^2)/D
    # y0 ~= rsqrt(D * n2m) = exp(-0.5*ln(n2m) - 0.5*ln(D))
    lnv = stat.tile([P, ng], F32, tag="lnv")
    nc.scalar.activation(lnv[:], n2m[:], AF.Ln)
    y0 = stat.tile([P, ng], F32, tag="y0")
    nc.scalar.activation(y0[:], lnv[:], AF.Exp, bias=lnb[:], scale=-0.5)
    # Newton: y1 = y0*(1.5 - 0.5*(D*n2m)*y0^2)
    t0 = stat.tile([P, ng], F32, tag="t0")
    nc.vector.tensor_mul(t0[:], y0[:], y0[:])
    t1 = stat.tile([P, ng], F32, tag="t1")
    nc.vector.tensor_mul(t1[:], t0[:], n2m[:])
    u = stat.tile([P, ng], F32, tag="u")
    nc.vector.tensor_scalar(u[:], t1[:], -0.5 * D, 1.5, op0=ALU.mult, op1=ALU.add)
    y1 = stat.tile([P, ng], F32, tag="y1")
    nc.vector.tensor_mul(y1[:], y0[:], u[:])

    # fused scale+cast: z[p,k,g,:] = x[p,g,k*128:...] * y1[p,g]  (broadcast AP)
    z = zbf.tile([P, KH, ng, P], BF16, tag="z")
    yb = y1[:, :, None].to_broadcast([P, ng, P])
    for k in range(KH):
        scale_eng.tensor_mul(z[:, k, :, :], x[:, :, k * P : (k + 1) * P], yb)
    return y1, z


def _transpose_chunk(nc, z, ng, dest, goff):
    """DMA-xbar block transpose: z [128, KH, ng, 128] bf16 (row-major rows) ->
    dest[:, k, (goff+g)*128 + p] = z[p, k, g, dd] i.e. D-major columns."""
    for k in range(KH):
        out_ap = dest[:, k, goff * P : (goff + ng) * P].rearrange(
            "p (b s) -> p b s", s=P
        )
        nc.sync.dma_start_transpose(out_ap, z[:, k, :, :])


def _emit(tc):
    nc = tc.nc
    pa = nc.dram_tensor("pa", [B, D], F32, kind="ExternalInput").ap()
    pb = nc.dram_tensor("pb", [B, D], F32, kind="ExternalInput").ap()
    la = nc.dram_tensor("la", [LROWS, D], F32, kind="ExternalInput").ap()
    lb = nc.dram_tensor("lb", [LROWS, D], F32, kind="ExternalInput").ap()
    terms_out = nc.dram_tensor("terms", [P, MCH], F32, kind="ExternalOutput").ap()
    pos_out = nc.dram_tensor("pos", [P, NG_LOC], F32, kind="ExternalOutput").ap()

    import contextlib

    with contextlib.ExitStack() as ctx:
        persist = ctx.enter_context(tc.tile_pool(name="persist", bufs=1))
        xin = ctx.enter_context(tc.tile_pool(name="xin", bufs=2))
        sqp = ctx.enter_context(tc.tile_pool(name="sqp", bufs=2))
        zbf = ctx.enter_context(tc.tile_pool(name="zbf", bufs=2))
        stat = ctx.enter_context(tc.tile_pool(name="stat", bufs=3))
        expsc = ctx.enter_context(tc.tile_pool(name="expsc", bufs=2))
        sacc_pool = ctx.enter_context(tc.tile_pool(name="sacc", bufs=4))
        pprod_pool = ctx.enter_context(tc.tile_pool(name="pprod", bufs=1))
        psum = ctx.enter_context(tc.tile_pool(name="psum", bufs=2, space="PSUM"))
        pools = (xin, sqp, zbf, stat)

        # persistent operands
        repsT_a = persist.tile([P, KH, B], BF16, tag="repsT_a")  # cols 0..4095
        repsT_b = persist.tile([P, KH, B], BF16, tag="repsT_b")  # cols 4096..8191
        lhsT = persist.tile([P, KH, LROWS], BF16, tag="lhsT")
        posb = persist.tile([P, NG_LOC], F32, tag="posb")
        lns = persist.tile([P, MCH], F32, tag="lns")
        nbias = persist.tile([P, 1], F32, tag="nbias")
        nc.vector.memset(nbias[:], -INV_T)
        lnb = persist.tile([P, 1], F32, tag="lnb")
        nc.vector.memset(lnb[:], -0.5 * math.log(D))

        # ---- all input loads up-front (one xbar-mode stretch, queues fill early)
        xl = xin.tile([P, 2 * NG_LOC, D], F32, tag="xl")
        nc.sync.dma_start(xl[:, 0:NG_LOC, :], la.rearrange("(g p) d -> p g d", p=P))
        nc.sync.dma_start(
            xl[:, NG_LOC : 2 * NG_LOC, :], lb.rearrange("(g p) d -> p g d", p=P)
        )
        xa = xin.tile([P, NG_FULL_HALF, D], F32, tag="x")
        nc.sync.dma_start(xa[:], pa.rearrange("(g p) d -> p g d", p=P))
        xb = xin.tile([P, NG_FULL_HALF, D], F32, tag="x")
        nc.sync.dma_start(xb[:], pb.rearrange("(g p) d -> p g d", p=P))

        # ---- local chunk: lhsT + positives ----
        y1l, zl = _normalize_chunk(nc, pools, xl, 2 * NG_LOC, lnb, nc.vector)
        # lhsT from the first NG_LOC groups (la): zl[:, k, 0:NG_LOC, :]
        for k in range(KH):
            nc.sync.dma_start_transpose(
                lhsT[:, k, :].rearrange("p (b s) -> p b s", s=P),
                zl[:, k, 0:NG_LOC, :],
            )
        # positives in fp32: praw[g] = sum_d la[g]*lb[g]; pos = praw*y1a*y1b
        praw = stat.tile([P, NG_LOC], F32, tag="praw")
        pprod = pprod_pool.tile([P, NG_LOC, D], F32, tag="pprod")
        nc.vector.tensor_mul(
            pprod[:], xl[:, 0:NG_LOC, :], xl[:, NG_LOC : 2 * NG_LOC, :]
        )
        nc.vector.reduce_sum(praw[:], pprod[:], axis=mybir.AxisListType.X)
        pp = stat.tile([P, NG_LOC], F32, tag="pp")
        nc.vector.tensor_mul(pp[:], praw[:], y1l[:, 0:NG_LOC])
        nc.vector.tensor_mul(posb[:], pp[:], y1l[:, NG_LOC : 2 * NG_LOC])

        # ---- full chunks: repsT (pb's scale pass runs on the idle GpSimd) ----
        for xf, dest, eng in ((xa, repsT_a, nc.vector), (xb, repsT_b, nc.gpsimd)):
            _, zf = _normalize_chunk(nc, pools, xf, NG_FULL_HALF, lnb, eng)
            _transpose_chunk(nc, zf, NG_FULL_HALF, dest, 0)

        # ---- main loop: sim chunks + fused exp row-sums ----
        # two passes so pass A (repsT_a) overlaps with pb's setup
        saccs = []
        for m in range(MCH):
            sacc_m = sacc_pool.tile([P, NBLK], F32, tag=f"sacc{m}", name=f"sacc{m}")
            saccs.append(sacc_m)
        for half, rT in ((0, repsT_a), (1, repsT_b)):
            for m in range(MCH):
                for nbl in range(NBLK // 2):
                    nb = half * (NBLK // 2) + nbl
                    ps = psum.tile([P, PSUM_W], F32, tag="ps")
                    for k in range(KH):
                        for nn in range(PSUM_W // 512):
                            col = nbl * PSUM_W + nn * 512
                            nc.tensor.matmul(
                                ps[:, nn * 512 : (nn + 1) * 512],
                                lhsT=lhsT[:, k, m * P : (m + 1) * P],
                                rhs=rT[:, k, col : col + 512],
                                start=(k == 0),
                                stop=(k == KH - 1),
                            )
                    eo = expsc.tile([P, PSUM_W], BF16, tag="eo")
                    nc.scalar.activation(
                        eo[:],
                        ps[:],
                        AF.Exp,
                        bias=nbias[:],
                        scale=INV_T,
                        accum_out=saccs[m][:, nb : nb + 1],
                    )
        for m in range(MCH):
            stot = stat.tile([P, 1], F32, tag="stot")
            nc.vector.reduce_sum(stot[:], saccs[m][:], axis=mybir.AxisListType.X)
            nc.scalar.activation(lns[:, m : m + 1], stot[:], AF.Ln)

        # terms = ln(s) + (1000 - 1000*pos)   [lse - pos/t = 1000 + ln(s) - 1000*pos]
        posq = stat.tile([P, MCH], F32, tag="posq")
        nc.vector.tensor_scalar(
            posq[:], posb[:], -INV_T, INV_T, op0=ALU.mult, op1=ALU.add
        )
        terms = stat.tile([P, MCH], F32, tag="terms")
        nc.vector.tensor_add(terms[:], lns[:], posq[:])
        nc.sync.dma_start(terms_out, terms[:])
        nc.sync.dma_start(pos_out, posb[:])


_CACHE = {}


def _get_nc():
    if "nc" not in _CACHE:
        nc = bacc.Bacc("TRN2", target_bir_lowering=False, debug=False)
        with tile.TileContext(nc) as tc:
            _emit(tc)
        nc.finalize()
        _CACHE["nc"] = nc
    return _CACHE["nc"]


last_results = None


def kernel(proj_1: np.ndarray, proj_2: np.ndarray):
    global last_results
    p1 = np.ascontiguousarray(proj_1, dtype=np.float32)
    p2 = np.ascontiguousarray(proj_2, dtype=np.float32)
    nc = _get_nc()
    in_maps = []
    for c in range(NCORES):
        if c < 4:
            la = p1[c * LROWS : (c + 1) * LROWS]
            lb = p2[c * LROWS : (c + 1) * LROWS]
        else:
            la = p2[(c - 4) * LROWS : (c - 3) * LROWS]
            lb = p1[(c - 4) * LROWS : (c - 3) * LROWS]
        in_maps.append(
            {
                "pa": p1,
                "pb": p2,
                "la": np.ascontiguousarray(la),
                "lb": np.ascontiguousarray(lb),
            }
        )
    res = run_bass_kernel_spmd(nc, in_maps, core_ids=list(range(NCORES)))
    last_results = res
    term_sum = 0.0
    pos_sum = 0.0
    # reference returns sum(concat([pos, pos])) = 2*sum(pos); summing every
    # core's slice counts each pos value exactly twice.
    for c in range(NCORES):
        term_sum += res.results[c]["terms"].astype(np.float64).sum()
        pos_sum += res.results[c]["pos"].astype(np.float64).sum()
    loss = term_sum / NROWS
    return (np.float32(loss), np.float32(pos_sum))
